# revision 24
# baseline (speedup 1.0000x reference)
"""Trainium2 Bass kernel for nn_DecoderRNN: 9-layer residual LSTM chain, one step.

Math (per reference):
  layer 0: cell(x,            h[0], c[0], W_ih1,     W_hh[0], b[0])
  layer 1: cell(h1,           h[1], c[1], W_ih[0],   W_hh[1], b[1])
  layer k: cell(h_k-1+h_k-2,  h[k], c[k], W_ih[k-1], W_hh[k], b[k])   k=2..8
  out = h9 @ W_out.T + b_out ; also returns stacked hs, cs.

gates g = W_cat @ [h_init[k]; inp] + b, where only the `inp` half depends on
the chain.  All data pre-arranged host-side to [128 partitions, free] layout:
  - 256-vectors v -> tile [128, 2], v[j*128+p] = tile[p, j]
  - gate vectors (1024, reordered [i,f,o,g]) -> [128, 8], g[c*128+p] = G[p, c]
"""

import numpy as np

H = 256
L = 9
NG = 4 * H
P = 128
N_CORES = 8
F32 = np.float32

# gate reorder: pytorch [i,f,g,o] -> ours [i,f,o,g] (sigmoid cols 0:6, tanh 6:8)
_PERM = np.concatenate(
    [np.arange(0, H), np.arange(H, 2 * H), np.arange(3 * H, 4 * H), np.arange(2 * H, 3 * H)]
)

_CACHE = {}

# Transposed-mode gate map: G8[p, c] receives gate-stream element p*8+c.
# col c -> (gate_sel, chunk j): [i0,i1,f0,f1,o0,o1,g0,g1] with pytorch sel order i,f,g,o.
_COLSEL = [(0, 0), (0, 1), (1, 0), (1, 1), (3, 0), (3, 1), (2, 0), (2, 1)]
_GMAP = np.empty((P, 8), np.int64)
for _c, (_sel, _j) in enumerate(_COLSEL):
    for _p in range(P):
        _GMAP[_p, _c] = _sel * H + _j * P + _p
_GPERM_STREAM = _GMAP.reshape(-1)  # stream position p*8+c -> original gate row
_GPERM_STAT = _GMAP.T.reshape(-1)  # stationary free idx mc*128+pm -> original row


def _vec_to_pf(v):
    """[..., 256] -> [128, cols] partition-major tile layout."""
    v = np.asarray(v, F32).reshape(-1, 2, P)
    return np.ascontiguousarray(v.transpose(2, 0, 1).reshape(P, -1))


def _build(reps=0):
    """Build the Bass program. reps>0 wraps the whole chain in a For_i loop
    (benchmark mode; computes the same thing reps times)."""
    import concourse.bacc as bacc
    import concourse.mybir as mybir
    import concourse.tile as tile
    from contextlib import ExitStack

    f32 = mybir.dt.float32
    nc = bacc.Bacc(name=f"decoder_rnn_r{reps}", num_devices=N_CORES)

    w_d = nc.dram_tensor("w", [L, P, 4096], f32, kind="ExternalInput")
    b_d = nc.dram_tensor("b", [P, L * 8], f32, kind="ExternalInput")
    h_d = nc.dram_tensor("h0", [P, L * 2], f32, kind="ExternalInput")
    c_d = nc.dram_tensor("c0", [P, L * 2], f32, kind="ExternalInput")
    x_d = nc.dram_tensor("x", [P, 2], f32, kind="ExternalInput")
    wo_d = nc.dram_tensor("wo", [P, 2], f32, kind="ExternalInput")
    bo_d = nc.dram_tensor("bo", [1, 1], f32, kind="ExternalInput")

    hs_d = nc.dram_tensor("hs", [P, L * 2], f32, kind="ExternalOutput")
    cs_d = nc.dram_tensor("cs", [P, L * 2], f32, kind="ExternalOutput")
    out_d = nc.dram_tensor("out", [1, 1], f32, kind="ExternalOutput")

    with tile.TileContext(nc) as tc:
        with (
            tc.tile_pool(name="wpool", bufs=3) as wpool,
            tc.tile_pool(name="small", bufs=1) as small,
            tc.tile_pool(name="pw", bufs=2) as pw,
            tc.tile_pool(name="psum", bufs=2, space="PSUM") as psum,
            tc.tile_pool(name="pso", bufs=1, space="PSUM") as pso,
        ):
            h_in = small.tile([P, L * 2], f32, tag="h_in")
            c_in = small.tile([P, L * 2], f32, tag="c_in")
            x_t = small.tile([P, 2], f32, tag="x_t")
            b_t = small.tile([P, L * 8], f32, tag="b_t")
            wo_t = small.tile([P, 2], f32, tag="wo_t")
            bo_t = small.tile([1, 1], f32, tag="bo_t")
            hs_acc = small.tile([P, L * 2], f32, tag="hs_acc")
            cs_acc = small.tile([P, L * 2], f32, tag="cs_acc")
            dummy = small.tile([P, 1], f32, tag="dummy")

            nc.sync.dma_start(h_in[:], h_d[:])
            nc.sync.dma_start(c_in[:], c_d[:])
            nc.sync.dma_start(x_t[:], x_d[:])
            nc.sync.dma_start(b_t[:], b_d[:])
            nc.sync.dma_start(wo_t[:], wo_d[:])
            nc.sync.dma_start(bo_t[:], bo_d[:])

            # prefetch the sigmoid/tanh ACT table at t=0
            nc.scalar.activation(dummy[:], x_t[:, 0:1], mybir.ActivationFunctionType.Sigmoid)

            def body():
                prev_h = None
                inp = None
                for k in range(L):
                    w = wpool.tile([P, 4096], f32, tag="w")
                    nc.sync.dma_start(w[:], w_d[k, :, :])

                    G = psum.tile([P, 8], f32, tag="G")
                    for mc in range(8):
                        for kc in range(4):
                            if kc < 2:
                                rhs = h_in[:, 2 * k + kc : 2 * k + kc + 1]
                            elif k == 0:
                                rhs = x_t[:, kc - 2 : kc - 1]
                            else:
                                rhs = inp[:, kc - 2 : kc - 1]
                            nc.tensor.matmul(
                                G[:, mc : mc + 1],
                                w[:, kc * 1024 + mc * P : kc * 1024 + (mc + 1) * P],
                                rhs,
                                start=(kc == 0),
                                stop=(kc == 3),
                            )

                    Gb = pw.tile([P, 8], f32, tag="Gb")
                    nc.vector.tensor_add(Gb[:], G[:], b_t[:, 8 * k : 8 * k + 8])
                    S = pw.tile([P, 6], f32, tag="S")
                    T = pw.tile([P, 2], f32, tag="T")
                    nc.scalar.activation(S[:], Gb[:, 0:6], mybir.ActivationFunctionType.Sigmoid)
                    nc.scalar.activation(T[:], Gb[:, 6:8], mybir.ActivationFunctionType.Tanh)
                    t1 = pw.tile([P, 2], f32, tag="t1")
                    t2 = pw.tile([P, 2], f32, tag="t2")
                    nc.vector.tensor_mul(t1[:], S[:, 2:4], c_in[:, 2 * k : 2 * k + 2])
                    nc.vector.tensor_mul(t2[:], S[:, 0:2], T[:])
                    cnew = cs_acc[:, 2 * k : 2 * k + 2]
                    nc.vector.tensor_add(cnew, t1[:], t2[:])
                    Tc = pw.tile([P, 2], f32, tag="Tc")
                    nc.scalar.activation(Tc[:], cnew, mybir.ActivationFunctionType.Tanh)
                    hnew = hs_acc[:, 2 * k : 2 * k + 2]
                    nc.vector.tensor_mul(hnew, S[:, 4:6], Tc[:])

                    if k == 0:
                        inp = hs_acc[:, 0:2]
                    else:
                        ninp = pw.tile([P, 2], f32, tag=f"inp{k & 1}")
                        nc.vector.tensor_add(ninp[:], hnew, prev_h)
                        inp = ninp
                    prev_h = hnew

                o_ps = pso.tile([1, 1], f32, tag="ops")
                for kc in range(2):
                    nc.tensor.matmul(
                        o_ps[:],
                        hs_acc[:, 2 * (L - 1) + kc : 2 * (L - 1) + kc + 1],
                        wo_t[:, kc : kc + 1],
                        start=(kc == 0),
                        stop=(kc == 1),
                    )
                o_sb = pw.tile([1, 1], f32, tag="osb")
                nc.vector.tensor_add(o_sb[:], o_ps[:], bo_t[:])

                nc.sync.dma_start(hs_d[:], hs_acc[:])
                nc.sync.dma_start(cs_d[:], cs_acc[:])
                nc.sync.dma_start(out_d[:], o_sb[:])

            if reps > 0:
                with tc.For_i(0, reps, 1):
                    body()
            else:
                body()

    nc.compile()
    return nc


def _build_v2(reps=0, with_ag=True, mm_mc=8, slim_pw=False, transposed=False):
    """Distributed build: cores 1-7 compute static gates (W_hh@h[k]+b) for
    layers 2-8 in [128,8] form, one AllGather ships them to core 0, which runs
    layers 0-1 full-style plus the dynamic chain for layers 2-8.
    reps>0 (benchmark mode) requires with_ag=False (collectives can't sit in
    control flow); the chain then reads its local (garbage) ag_out."""
    import concourse.bacc as bacc
    import concourse.mybir as mybir
    import concourse.tile as tile

    assert reps == 0 or not with_ag
    f32 = mybir.dt.float32
    ACT = mybir.ActivationFunctionType
    nc = bacc.Bacc(
        name=f"decoder_v2_r{reps}_{int(with_ag)}_{mm_mc}_{int(slim_pw)}_{int(transposed)}",
        num_devices=N_CORES,
    )

    wcat01_d = nc.dram_tensor("wcat01", [2, P, 4096], f32, kind="ExternalInput")
    wdyn_d = nc.dram_tensor("wdyn", [L - 2, P, 2048], f32, kind="ExternalInput")
    wst_d = nc.dram_tensor("wst", [P, 2048], f32, kind="ExternalInput")
    bst_d = nc.dram_tensor("bst", [P, 8], f32, kind="ExternalInput")
    hslot_d = nc.dram_tensor("hslot", [P, 2], f32, kind="ExternalInput")
    b_d = nc.dram_tensor("b", [P, L * 8], f32, kind="ExternalInput")
    h_d = nc.dram_tensor("h0", [P, L * 2], f32, kind="ExternalInput")
    c_d = nc.dram_tensor("c0", [P, L * 2], f32, kind="ExternalInput")
    x_d = nc.dram_tensor("x", [P, 2], f32, kind="ExternalInput")
    wo_d = nc.dram_tensor("wo", [P, 2], f32, kind="ExternalInput")
    bo_d = nc.dram_tensor("bo", [1, 1], f32, kind="ExternalInput")

    hs_d = nc.dram_tensor("hs", [P, L * 2], f32, kind="ExternalOutput")
    cs_d = nc.dram_tensor("cs", [P, L * 2], f32, kind="ExternalOutput")
    out_d = nc.dram_tensor("out", [1, 1], f32, kind="ExternalOutput")

    with tile.TileContext(nc) as tc:
        with (
            tc.tile_pool(name="wpool", bufs=3) as wpool,
            tc.tile_pool(name="wdpool", bufs=4) as wdpool,
            tc.tile_pool(name="small", bufs=1) as small,
            tc.tile_pool(name="gst", bufs=L - 2) as gstp,
            tc.tile_pool(name="pw", bufs=2) as pw,
            tc.tile_pool(name="psum", bufs=2, space="PSUM") as psum,
            tc.tile_pool(name="pso", bufs=1, space="PSUM") as pso,
            tc.tile_pool(name="dram", bufs=1, space="DRAM") as dram,
        ):
            h_in = small.tile([P, L * 2], f32, tag="h_in")
            c_in = small.tile([P, L * 2], f32, tag="c_in")
            x_t = small.tile([P, 2], f32, tag="x_t")
            b_t = small.tile([P, L * 8], f32, tag="b_t")
            wo_t = small.tile([P, 2], f32, tag="wo_t")
            bo_t = small.tile([1, 1], f32, tag="bo_t")
            hs_acc = small.tile([P, L * 2], f32, tag="hs_acc")
            cs_acc = small.tile([P, L * 2], f32, tag="cs_acc")
            dummy = small.tile([P, 1], f32, tag="dummy")

            nc.sync.dma_start(h_in[:], h_d[:])
            nc.sync.dma_start(c_in[:], c_d[:])
            nc.sync.dma_start(x_t[:], x_d[:])
            nc.sync.dma_start(b_t[:], b_d[:])
            nc.sync.dma_start(wo_t[:], wo_d[:])
            nc.sync.dma_start(bo_t[:], bo_d[:])
            nc.scalar.activation(dummy[:], x_t[:, 0:1], ACT.Sigmoid)

            ag_in = dram.tile([P, 8], f32, tag="ag_in")
            ag_out = dram.tile([N_CORES * P, 8], f32, tag="ag_out")

            pid = nc.partition_id()

            # ---- producers: static gates for layer (pid+1), pid=1..7 ----
            with tc.If(pid != 0):
                wst = small.tile([P, 2048], f32, tag="wst")
                bst = small.tile([P, 8], f32, tag="bst")
                hsl = small.tile([P, 2], f32, tag="hsl")
                nc.sync.dma_start(wst[:], wst_d[:])
                nc.sync.dma_start(bst[:], bst_d[:])
                nc.sync.dma_start(hsl[:], hslot_d[:])
                Gs = psum.tile([P, 8], f32, tag="Gs")
                for mc in range(8):
                    for kc in range(2):
                        nc.tensor.matmul(
                            Gs[:, mc : mc + 1],
                            wst[:, kc * 1024 + mc * P : kc * 1024 + (mc + 1) * P],
                            hsl[:, kc : kc + 1],
                            start=(kc == 0),
                            stop=(kc == 1),
                        )
                Gsb = pw.tile([P, 8], f32, tag="Gsb")
                nc.vector.tensor_add(Gsb[:], Gs[:], bst[:])
                nc.sync.dma_start(ag_in[:], Gsb[:])

            if with_ag:
                nc.gpsimd.collective_compute(
                    "AllGather",
                    mybir.AluOpType.bypass,
                    replica_groups=[list(range(N_CORES))],
                    ins=[ag_in.opt()],
                    outs=[ag_out.opt()],
                )

            # ---- core 0: the chain ----
            with tc.If(pid == 0):

                def chain():
                    # prefetch static gate tiles for layers 2..8 (slots 1..7)
                    gst = []
                    for j in range(L - 2):
                        g = gstp.tile([P, 8], f32, tag="g")
                        nc.sync.dma_start(g[:], ag_out[(j + 1) * P : (j + 2) * P, :])
                        gst.append(g)

                    prev_h = None
                    inp = None
                    for k in range(L):
                        if k < 2:
                            w = wpool.tile([P, 4096], f32, tag="w")
                            nc.sync.dma_start(w[:], wcat01_d[k, :, :])
                        else:
                            w = wdpool.tile([P, 2048], f32, tag="wd")
                            nc.sync.dma_start(w[:], wdyn_d[k - 2, :, :])

                        nkc = 4 if k < 2 else 2

                        def vec(kc):
                            if k < 2:
                                if kc < 2:
                                    return h_in[:, 2 * k + kc : 2 * k + kc + 1]
                                elif k == 0:
                                    return x_t[:, kc - 2 : kc - 1]
                                else:
                                    return inp[:, kc - 2 : kc - 1]
                            return inp[:, kc : kc + 1]

                        if transposed:
                            # gates^T in two [1,512] psum halves; weights stream as rhs
                            g1s = pw.tile([1, 1024], f32, tag="g1s")
                            for nh in range(2):
                                Gh = psum.tile([1, 512], f32, tag="Gh")
                                for kc in range(nkc):
                                    nc.tensor.matmul(
                                        Gh[:],
                                        vec(kc),
                                        w[:, kc * 1024 + nh * 512 : kc * 1024 + nh * 512 + 512],
                                        start=(kc == 0),
                                        stop=(kc == nkc - 1),
                                    )
                                nc.scalar.activation(
                                    g1s[:, nh * 512 : (nh + 1) * 512], Gh[:], ACT.Copy
                                )
                            G = pw.tile([P, 8], f32, tag="G8")
                            nc.sync.dma_start(G[:], g1s[:, :])
                        else:
                            G = psum.tile([P, 8], f32, tag="G")
                            for mc in range(mm_mc):
                                for kc in range(nkc):
                                    nc.tensor.matmul(
                                        G[:, mc : mc + 1],
                                        w[:, kc * 1024 + mc * P : kc * 1024 + (mc + 1) * P],
                                        vec(kc),
                                        start=(kc == 0),
                                        stop=(kc == nkc - 1),
                                    )

                        Gb = pw.tile([P, 8], f32, tag="Gb")
                        if k < 2:
                            nc.vector.tensor_add(Gb[:], G[:], b_t[:, 8 * k : 8 * k + 8])
                        else:
                            nc.vector.tensor_add(Gb[:], G[:], gst[k - 2][:])
                        if slim_pw:
                            Sx = pw.tile([P, 8], f32, tag="Sx")
                            nc.scalar.activation(Sx[:], Gb[:], ACT.Sigmoid)
                            cnew = cs_acc[:, 2 * k : 2 * k + 2]
                            nc.vector.tensor_mul(cnew, Sx[:, 2:4], c_in[:, 2 * k : 2 * k + 2])
                            hnew = hs_acc[:, 2 * k : 2 * k + 2]
                            nc.vector.tensor_mul(hnew, Sx[:, 4:6], cnew)
                            if k == 0:
                                inp = hs_acc[:, 0:2]
                            else:
                                ninp = pw.tile([P, 2], f32, tag=f"inp{k & 1}")
                                nc.vector.tensor_add(ninp[:], hnew, prev_h)
                                inp = ninp
                            prev_h = hnew
                            continue
                        S = pw.tile([P, 6], f32, tag="S")
                        T = pw.tile([P, 2], f32, tag="T")
                        nc.scalar.activation(S[:], Gb[:, 0:6], ACT.Sigmoid)
                        nc.scalar.activation(T[:], Gb[:, 6:8], ACT.Tanh)
                        t1 = pw.tile([P, 2], f32, tag="t1")
                        t2 = pw.tile([P, 2], f32, tag="t2")
                        nc.vector.tensor_mul(t1[:], S[:, 2:4], c_in[:, 2 * k : 2 * k + 2])
                        nc.vector.tensor_mul(t2[:], S[:, 0:2], T[:])
                        cnew = cs_acc[:, 2 * k : 2 * k + 2]
                        nc.vector.tensor_add(cnew, t1[:], t2[:])
                        Tc = pw.tile([P, 2], f32, tag="Tc")
                        nc.scalar.activation(Tc[:], cnew, ACT.Tanh)
                        hnew = hs_acc[:, 2 * k : 2 * k + 2]
                        nc.vector.tensor_mul(hnew, S[:, 4:6], Tc[:])

                        if k == 0:
                            inp = hs_acc[:, 0:2]
                        else:
                            ninp = pw.tile([P, 2], f32, tag=f"inp{k & 1}")
                            nc.vector.tensor_add(ninp[:], hnew, prev_h)
                            inp = ninp
                        prev_h = hnew

                    o_ps = pso.tile([1, 1], f32, tag="ops")
                    for kc in range(2):
                        nc.tensor.matmul(
                            o_ps[:],
                            hs_acc[:, 2 * (L - 1) + kc : 2 * (L - 1) + kc + 1],
                            wo_t[:, kc : kc + 1],
                            start=(kc == 0),
                            stop=(kc == 1),
                        )
                    o_sb = pw.tile([1, 1], f32, tag="osb")
                    nc.vector.tensor_add(o_sb[:], o_ps[:], bo_t[:])

                    nc.sync.dma_start(hs_d[:], hs_acc[:])
                    nc.sync.dma_start(cs_d[:], cs_acc[:])
                    nc.sync.dma_start(out_d[:], o_sb[:])

                if reps > 0:
                    with tc.For_i(0, reps, 1):
                        chain()
                else:
                    chain()

    nc.compile()
    return nc


def _prep_inputs_v2(x, h, c, W_ih1, W_ih, W_hh, b_ih, b_hh, W_out, b_out, transposed=False):
    """Returns list of 8 per-core input dicts."""
    base = _prep_inputs(x, h, c, W_ih1, W_ih, W_hh, b_ih, b_hh, W_out, b_out)
    W_ih = np.asarray(W_ih, F32)
    W_ih1 = np.asarray(W_ih1, F32)
    W_hh = np.asarray(W_hh, F32)
    b_raw = np.asarray(b_ih, F32) + np.asarray(b_hh, F32)  # [9, 1024] unpermuted
    b = b_raw[:, _PERM]
    h2 = np.asarray(h, F32).reshape(L, H)

    if transposed:
        # moving-weight layout: w[p, kc*1024 + nh*512 + n] = W[gperm[nh*512+n], kc*128+p]
        wcat01 = np.zeros((2, P, 4096), F32)
        for k in range(2):
            wcat = np.zeros((NG, 2 * H), F32)
            wcat[:, :H] = W_hh[k]
            if k == 0:
                wcat[:, H : H + 2] = W_ih1
            else:
                wcat[:, H:] = W_ih[0]
            wp = wcat[_GPERM_STREAM, :]  # [1024, 512]
            wcat01[k] = wp.T.reshape(4, P, 2, 512).transpose(1, 0, 2, 3).reshape(P, 4096)
        wdyn = np.zeros((L - 2, P, 2048), F32)
        for k in range(2, L):
            wp = W_ih[k - 1][_GPERM_STREAM, :]  # [1024, 256]
            wdyn[k - 2] = wp.T.reshape(2, P, 2, 512).transpose(1, 0, 2, 3).reshape(P, 2048)
        base = dict(base)
        barr = b_raw[:, _GMAP]  # [9, 128, 8]
        base["b"] = np.ascontiguousarray(barr.transpose(1, 0, 2).reshape(P, L * 8))
    else:
        wcat01 = base["w"][0:2]  # [2,128,4096]
        wdyn = np.zeros((L - 2, P, 2048), F32)
        for k in range(2, L):
            wp = W_ih[k - 1][_PERM, :]  # [1024, 256]
            wdyn[k - 2] = wp.T.reshape(2, P, 8, P).transpose(1, 0, 2, 3).reshape(P, 2048)

    zeros_wst = np.zeros((P, 2048), F32)
    zeros_b8 = np.zeros((P, 8), F32)
    zeros_h2 = np.zeros((P, 2), F32)
    maps = []
    for cid in range(N_CORES):
        m = {
            "wcat01": wcat01 if cid == 0 else np.zeros_like(wcat01),
            "wdyn": wdyn if cid == 0 else np.zeros_like(wdyn),
            "b": base["b"],
            "h0": base["h0"],
            "c0": base["c0"],
            "x": base["x"],
            "wo": base["wo"],
            "bo": base["bo"],
        }
        if cid == 0:
            m["wst"] = zeros_wst
            m["bst"] = zeros_b8
            m["hslot"] = zeros_h2
        else:
            k = cid + 1  # layer produced by this core
            rowsel = _GPERM_STAT if transposed else _PERM
            wp = W_hh[k][rowsel, :]  # [1024, 256]
            m["wst"] = np.ascontiguousarray(
                wp.T.reshape(2, P, 8, P).transpose(1, 0, 2, 3).reshape(P, 2048)
            )
            if transposed:
                m["bst"] = np.ascontiguousarray(b_raw[k][_GMAP])
            else:
                m["bst"] = np.ascontiguousarray(b[k].reshape(8, P).T)
            m["hslot"] = _vec_to_pf(h2[k])
        maps.append(m)
    return maps


def _prep_inputs(x, h, c, W_ih1, W_ih, W_hh, b_ih, b_hh, W_out, b_out):
    x = np.asarray(x, F32).reshape(2)
    h = np.asarray(h, F32).reshape(L, H)
    c = np.asarray(c, F32).reshape(L, H)
    W_ih1 = np.asarray(W_ih1, F32)
    W_ih = np.asarray(W_ih, F32)
    W_hh = np.asarray(W_hh, F32)
    b = (np.asarray(b_ih, F32) + np.asarray(b_hh, F32))[:, _PERM]  # [9, 1024]
    W_out = np.asarray(W_out, F32).reshape(1, H)
    b_out = np.asarray(b_out, F32).reshape(1)

    w_host = np.zeros((L, P, 4096), F32)
    for k in range(L):
        wcat = np.zeros((NG, 2 * H), F32)
        wcat[:, :H] = W_hh[k]
        if k == 0:
            wcat[:, H : H + 2] = W_ih1
        else:
            wcat[:, H:] = W_ih[k - 1]
        wp = wcat[_PERM, :]  # [1024, 512]
        lt = wp.T.reshape(4, P, 8, P)  # [kc, p, mc, pm]
        w_host[k] = lt.transpose(1, 0, 2, 3).reshape(P, 4096)

    xp = np.zeros(H, F32)
    xp[:2] = x

    return {
        "w": w_host,
        "b": np.ascontiguousarray(b.reshape(L, 8, P).transpose(2, 0, 1).reshape(P, L * 8)),
        "h0": _vec_to_pf(h),
        "c0": _vec_to_pf(c),
        "x": _vec_to_pf(xp),
        "wo": _vec_to_pf(W_out[0]),
        "bo": b_out.reshape(1, 1),
    }


def _postprocess(r0):
    hs = np.asarray(r0["hs"])  # [128, 18]
    cs = np.asarray(r0["cs"])
    out = np.asarray(r0["out"]).reshape(1, 1)
    hs_full = hs.reshape(P, L, 2).transpose(1, 2, 0).reshape(L, 1, 1, H)
    cs_full = cs.reshape(P, L, 2).transpose(1, 2, 0).reshape(L, 1, 1, H)
    return (
        np.ascontiguousarray(out),
        np.ascontiguousarray(hs_full),
        np.ascontiguousarray(cs_full),
    )


def make_runner(reps=0, ver="v1", with_ag=True, mm_mc=8, slim_pw=False, transposed=False):
    """Build (cached) and return run(in_maps_list) -> list of per-core result dicts.
    Keeps a persistent jitted callable so repeated calls avoid retracing."""
    import jax
    import numpy as _np
    from jax.sharding import Mesh, PartitionSpec
    from jax.experimental.shard_map import shard_map
    import concourse.mybir as mybir
    from concourse import bass2jax

    key = ("runner", reps, ver, with_ag, mm_mc, slim_pw, transposed)
    if key in _CACHE:
        return _CACHE[key]

    nc = _build(reps) if ver == "v1" else _build_v2(reps, with_ag, mm_mc, slim_pw, transposed)
    bass2jax.install_neuronx_cc_hook()

    partition_name = nc.partition_id_tensor.name if nc.partition_id_tensor else None
    in_names, out_names, out_avals, zero_outs = [], [], [], []
    for alloc in nc.m.functions[0].allocations:
        if not isinstance(alloc, mybir.MemoryLocationSet):
            continue
        name = alloc.memorylocations[0].name
        if alloc.kind == "ExternalInput":
            if name != partition_name:
                in_names.append(name)
        elif alloc.kind == "ExternalOutput":
            shape = tuple(alloc.tensor_shape)
            dtype = mybir.dt.np(alloc.dtype)
            out_names.append(name)
            out_avals.append(jax.core.ShapedArray(shape, dtype))
            zero_outs.append(_np.zeros(shape, dtype))
    n_params = len(in_names)
    n_outs = len(out_avals)
    all_in_names = in_names + out_names + ([partition_name] if partition_name else [])
    donate = tuple(range(n_params, n_params + n_outs))

    def _body(*args):
        operands = list(args)
        if partition_name is not None:
            operands.append(bass2jax.partition_id_tensor())
        outs = bass2jax._bass_exec_p.bind(
            *operands,
            out_avals=tuple(out_avals),
            in_names=tuple(all_in_names),
            out_names=tuple(out_names),
            lowering_input_output_aliases=(),
            sim_require_finite=True,
            sim_require_nnan=True,
            nc=nc,
        )
        return tuple(outs)

    devices = jax.devices()[:N_CORES]
    mesh = Mesh(_np.asarray(devices), ("core",))
    in_specs = (PartitionSpec("core"),) * (n_params + n_outs)
    out_specs = (PartitionSpec("core"),) * n_outs
    sharded = jax.jit(
        shard_map(_body, mesh=mesh, in_specs=in_specs, out_specs=out_specs, check_rep=False),
        donate_argnums=donate,
        keep_unused=True,
    )

    state = {}

    def run(in_maps, reuse_inputs=False):
        if not reuse_inputs or "dev_in" not in state:
            concat_in = [
                _np.concatenate([_np.asarray(m[nm]) for m in in_maps], axis=0)
                for nm in in_names
            ]
            state["dev_in"] = [jax.device_put(a) for a in concat_in]
        concat_zeros = [
            _np.zeros((N_CORES * z.shape[0], *z.shape[1:]), z.dtype) for z in zero_outs
        ]
        out_arrs = sharded(*state["dev_in"], *concat_zeros)
        jax.block_until_ready(out_arrs)
        return [
            {
                nm: _np.asarray(out_arrs[i]).reshape(N_CORES, *out_avals[i].shape)[c]
                for i, nm in enumerate(out_names)
            }
            for c in range(N_CORES)
        ], out_arrs

    _CACHE[key] = run
    return run


def kernel(x, h, c, W_ih1, W_ih, W_hh, b_ih, b_hh, W_out, b_out):
    in_maps = _prep_inputs_v2(x, h, c, W_ih1, W_ih, W_hh, b_ih, b_hh, W_out, b_out)
    run = make_runner(0, ver="v2", with_ag=True)
    results, _ = run(in_maps)
    return _postprocess(results[0])


# revision 44
# speedup vs baseline: 1.8396x; 1.8396x over previous
"""Trainium2 Bass kernel for nn_DecoderRNN: 9-layer residual LSTM chain, one step.

Math (per reference):
  layer 0: cell(x,            h[0], c[0], W_ih1,     W_hh[0], b[0])
  layer 1: cell(h1,           h[1], c[1], W_ih[0],   W_hh[1], b[1])
  layer k: cell(h_k-1+h_k-2,  h[k], c[k], W_ih[k-1], W_hh[k], b[k])   k=2..8
  out = h9 @ W_out.T + b_out ; also returns stacked hs, cs.

gates g = W_cat @ [h_init[k]; inp] + b, where only the `inp` half depends on
the chain.  All data pre-arranged host-side to [128 partitions, free] layout:
  - 256-vectors v -> tile [128, 2], v[j*128+p] = tile[p, j]
  - gate vectors (1024, reordered [i,f,o,g]) -> [128, 8], g[c*128+p] = G[p, c]
"""

import numpy as np

H = 256
L = 9
NG = 4 * H
P = 128
N_CORES = 8
F32 = np.float32

# gate reorder: pytorch [i,f,g,o] -> ours [i,f,o,g] (sigmoid cols 0:6, tanh 6:8)
_PERM = np.concatenate(
    [np.arange(0, H), np.arange(H, 2 * H), np.arange(3 * H, 4 * H), np.arange(2 * H, 3 * H)]
)

_CACHE = {}

# Transposed-mode gate map: G8[p, c] receives gate-stream element p*8+c.
# col c -> (gate_sel, chunk j): [i0,i1,f0,f1,o0,o1,g0,g1] with pytorch sel order i,f,g,o.
_COLSEL = [(0, 0), (0, 1), (1, 0), (1, 1), (3, 0), (3, 1), (2, 0), (2, 1)]
_GMAP = np.empty((P, 8), np.int64)
for _c, (_sel, _j) in enumerate(_COLSEL):
    for _p in range(P):
        _GMAP[_p, _c] = _sel * H + _j * P + _p
_GPERM_STREAM = _GMAP.reshape(-1)  # stream position p*8+c -> original gate row
_GPERM_STAT = _GMAP.T.reshape(-1)  # stationary free idx mc*128+pm -> original row


def _vec_to_pf(v):
    """[..., 256] -> [128, cols] partition-major tile layout."""
    v = np.asarray(v, F32).reshape(-1, 2, P)
    return np.ascontiguousarray(v.transpose(2, 0, 1).reshape(P, -1))


def _build(reps=0):
    """Build the Bass program. reps>0 wraps the whole chain in a For_i loop
    (benchmark mode; computes the same thing reps times)."""
    import concourse.bacc as bacc
    import concourse.mybir as mybir
    import concourse.tile as tile
    from contextlib import ExitStack

    f32 = mybir.dt.float32
    nc = bacc.Bacc(name=f"decoder_rnn_r{reps}", num_devices=N_CORES)

    w_d = nc.dram_tensor("w", [L, P, 4096], f32, kind="ExternalInput")
    b_d = nc.dram_tensor("b", [P, L * 8], f32, kind="ExternalInput")
    h_d = nc.dram_tensor("h0", [P, L * 2], f32, kind="ExternalInput")
    c_d = nc.dram_tensor("c0", [P, L * 2], f32, kind="ExternalInput")
    x_d = nc.dram_tensor("x", [P, 2], f32, kind="ExternalInput")
    wo_d = nc.dram_tensor("wo", [P, 2], f32, kind="ExternalInput")
    bo_d = nc.dram_tensor("bo", [1, 1], f32, kind="ExternalInput")

    hs_d = nc.dram_tensor("hs", [P, L * 2], f32, kind="ExternalOutput")
    cs_d = nc.dram_tensor("cs", [P, L * 2], f32, kind="ExternalOutput")
    out_d = nc.dram_tensor("out", [1, 1], f32, kind="ExternalOutput")

    with tile.TileContext(nc) as tc:
        with (
            tc.tile_pool(name="wpool", bufs=3) as wpool,
            tc.tile_pool(name="small", bufs=1) as small,
            tc.tile_pool(name="pw", bufs=2) as pw,
            tc.tile_pool(name="psum", bufs=2, space="PSUM") as psum,
            tc.tile_pool(name="pso", bufs=1, space="PSUM") as pso,
        ):
            h_in = small.tile([P, L * 2], f32, tag="h_in")
            c_in = small.tile([P, L * 2], f32, tag="c_in")
            x_t = small.tile([P, 2], f32, tag="x_t")
            b_t = small.tile([P, L * 8], f32, tag="b_t")
            wo_t = small.tile([P, 2], f32, tag="wo_t")
            bo_t = small.tile([1, 1], f32, tag="bo_t")
            hs_acc = small.tile([P, L * 2], f32, tag="hs_acc")
            cs_acc = small.tile([P, L * 2], f32, tag="cs_acc")
            dummy = small.tile([P, 1], f32, tag="dummy")

            nc.sync.dma_start(h_in[:], h_d[:])
            nc.sync.dma_start(c_in[:], c_d[:])
            nc.sync.dma_start(x_t[:], x_d[:])
            nc.sync.dma_start(b_t[:], b_d[:])
            nc.sync.dma_start(wo_t[:], wo_d[:])
            nc.sync.dma_start(bo_t[:], bo_d[:])

            # prefetch the sigmoid/tanh ACT table at t=0
            nc.scalar.activation(dummy[:], x_t[:, 0:1], mybir.ActivationFunctionType.Sigmoid)

            def body():
                prev_h = None
                inp = None
                for k in range(L):
                    w = wpool.tile([P, 4096], f32, tag="w")
                    nc.sync.dma_start(w[:], w_d[k, :, :])

                    G = psum.tile([P, 8], f32, tag="G")
                    for mc in range(8):
                        for kc in range(4):
                            if kc < 2:
                                rhs = h_in[:, 2 * k + kc : 2 * k + kc + 1]
                            elif k == 0:
                                rhs = x_t[:, kc - 2 : kc - 1]
                            else:
                                rhs = inp[:, kc - 2 : kc - 1]
                            nc.tensor.matmul(
                                G[:, mc : mc + 1],
                                w[:, kc * 1024 + mc * P : kc * 1024 + (mc + 1) * P],
                                rhs,
                                start=(kc == 0),
                                stop=(kc == 3),
                            )

                    Gb = pw.tile([P, 8], f32, tag="Gb")
                    nc.vector.tensor_add(Gb[:], G[:], b_t[:, 8 * k : 8 * k + 8])
                    S = pw.tile([P, 6], f32, tag="S")
                    T = pw.tile([P, 2], f32, tag="T")
                    nc.scalar.activation(S[:], Gb[:, 0:6], mybir.ActivationFunctionType.Sigmoid)
                    nc.scalar.activation(T[:], Gb[:, 6:8], mybir.ActivationFunctionType.Tanh)
                    t1 = pw.tile([P, 2], f32, tag="t1")
                    t2 = pw.tile([P, 2], f32, tag="t2")
                    nc.vector.tensor_mul(t1[:], S[:, 2:4], c_in[:, 2 * k : 2 * k + 2])
                    nc.vector.tensor_mul(t2[:], S[:, 0:2], T[:])
                    cnew = cs_acc[:, 2 * k : 2 * k + 2]
                    nc.vector.tensor_add(cnew, t1[:], t2[:])
                    Tc = pw.tile([P, 2], f32, tag="Tc")
                    nc.scalar.activation(Tc[:], cnew, mybir.ActivationFunctionType.Tanh)
                    hnew = hs_acc[:, 2 * k : 2 * k + 2]
                    nc.vector.tensor_mul(hnew, S[:, 4:6], Tc[:])

                    if k == 0:
                        inp = hs_acc[:, 0:2]
                    else:
                        ninp = pw.tile([P, 2], f32, tag=f"inp{k & 1}")
                        nc.vector.tensor_add(ninp[:], hnew, prev_h)
                        inp = ninp
                    prev_h = hnew

                o_ps = pso.tile([1, 1], f32, tag="ops")
                for kc in range(2):
                    nc.tensor.matmul(
                        o_ps[:],
                        hs_acc[:, 2 * (L - 1) + kc : 2 * (L - 1) + kc + 1],
                        wo_t[:, kc : kc + 1],
                        start=(kc == 0),
                        stop=(kc == 1),
                    )
                o_sb = pw.tile([1, 1], f32, tag="osb")
                nc.vector.tensor_add(o_sb[:], o_ps[:], bo_t[:])

                nc.sync.dma_start(hs_d[:], hs_acc[:])
                nc.sync.dma_start(cs_d[:], cs_acc[:])
                nc.sync.dma_start(out_d[:], o_sb[:])

            if reps > 0:
                with tc.For_i(0, reps, 1):
                    body()
            else:
                body()

    nc.compile()
    return nc


def _build_v3(reps=0, with_ag=True, col_tile=0, prescale=False):
    """Like v2 but layer 0 (fully static; W_ih1@x folded into bias host-side)
    and layer 1's static half are produced off-core too.  Producers hold two
    slots; two AllGathers so early layers aren't gated on late producers.
    Core 0 chain: L0 = pointwise only; L1-L8 = 16 MM pairs + pointwise each."""
    import concourse.bacc as bacc
    import concourse.mybir as mybir
    import concourse.tile as tile

    assert reps == 0 or not with_ag
    f32 = mybir.dt.float32
    ACT = mybir.ActivationFunctionType
    nc = bacc.Bacc(
        name=f"decoder_v3_r{reps}_{int(with_ag)}_{col_tile}_{int(prescale)}",
        num_devices=N_CORES,
    )

    wdyn_d = nc.dram_tensor("wdyn", [L - 1, P, 2048], f32, kind="ExternalInput")
    wst_d = nc.dram_tensor("wst", [2, P, 2048], f32, kind="ExternalInput")
    bst_d = nc.dram_tensor("bst", [2, P, 8], f32, kind="ExternalInput")
    hslot_d = nc.dram_tensor("hslot", [2, P, 2], f32, kind="ExternalInput")
    h_d = nc.dram_tensor("h0", [P, L * 2], f32, kind="ExternalInput")
    c_d = nc.dram_tensor("c0", [P, L * 2], f32, kind="ExternalInput")
    x_d = nc.dram_tensor("x", [P, 2], f32, kind="ExternalInput")
    wo_d = nc.dram_tensor("wo", [P, 2], f32, kind="ExternalInput")
    bo_d = nc.dram_tensor("bo", [1, 1], f32, kind="ExternalInput")

    hs_d = nc.dram_tensor("hs", [P, L * 2], f32, kind="ExternalOutput")
    cs_d = nc.dram_tensor("cs", [P, L * 2], f32, kind="ExternalOutput")
    out_d = nc.dram_tensor("out", [1, 1], f32, kind="ExternalOutput")

    with tile.TileContext(nc) as tc:
        with (
            tc.tile_pool(name="wdpool", bufs=4) as wdpool,
            tc.tile_pool(name="small", bufs=1) as small,
            tc.tile_pool(name="gst", bufs=L) as gstp,
            tc.tile_pool(name="pw", bufs=2) as pw,
            tc.tile_pool(name="psum", bufs=2, space="PSUM") as psum,
            tc.tile_pool(name="pso", bufs=1, space="PSUM") as pso,
            tc.tile_pool(name="dram", bufs=1, space="DRAM") as dram,
        ):
            h_in = small.tile([P, L * 2], f32, tag="h_in")
            c_in = small.tile([P, L * 2], f32, tag="c_in")
            x_t = small.tile([P, 2], f32, tag="x_t")
            wo_t = small.tile([P, 2], f32, tag="wo_t")
            bo_t = small.tile([1, 1], f32, tag="bo_t")
            hs_acc = small.tile([P, L * 2], f32, tag="hs_acc")
            cs_acc = small.tile([P, L * 2], f32, tag="cs_acc")
            dummy = small.tile([P, 1], f32, tag="dummy")

            nc.sync.dma_start(h_in[:], h_d[:])
            nc.sync.dma_start(c_in[:], c_d[:])
            nc.sync.dma_start(x_t[:], x_d[:])
            nc.sync.dma_start(wo_t[:], wo_d[:])
            nc.sync.dma_start(bo_t[:], bo_d[:])
            nc.scalar.activation(dummy[:], x_t[:, 0:1], ACT.Sigmoid)

            ag_inA = dram.tile([P, 8], f32, tag="ag_inA")
            ag_inB = dram.tile([P, 8], f32, tag="ag_inB")
            ag_outA = dram.tile([N_CORES * P, 8], f32, tag="ag_outA")
            ag_outB = dram.tile([N_CORES * P, 8], f32, tag="ag_outB")

            pid = nc.partition_id()

            with tc.If(pid != 0):
                for s, ag_in in ((0, ag_inA), (1, ag_inB)):
                    wst = small.tile([P, 2048], f32, tag=f"wst{s}")
                    bst = small.tile([P, 8], f32, tag=f"bst{s}")
                    hsl = small.tile([P, 2], f32, tag=f"hsl{s}")
                    nc.sync.dma_start(wst[:], wst_d[s, :, :])
                    nc.sync.dma_start(bst[:], bst_d[s, :, :])
                    nc.sync.dma_start(hsl[:], hslot_d[s, :, :])
                    Gs = psum.tile([P, 8], f32, tag="Gs")
                    for mc in range(8):
                        for kc in range(2):
                            for j in range(2):
                                base = kc * 1024 + mc * P + j * 64
                                nc.tensor.matmul(
                                    Gs[j * 64 : (j + 1) * 64, mc : mc + 1],
                                    wst[:, base : base + 64],
                                    hsl[:, kc : kc + 1],
                                    start=(kc == 0),
                                    stop=(kc == 1),
                                    tile_position=(0, j * 64),
                                )
                    Gsb = pw.tile([P, 8], f32, tag="Gsb")
                    nc.vector.tensor_add(Gsb[:], Gs[:], bst[:])
                    nc.sync.dma_start(ag_in[:], Gsb[:])

            if with_ag:
                for ag_in, ag_out in ((ag_inA, ag_outA), (ag_inB, ag_outB)):
                    nc.gpsimd.collective_compute(
                        "AllGather",
                        mybir.AluOpType.bypass,
                        replica_groups=[list(range(N_CORES))],
                        ins=[ag_in.opt()],
                        outs=[ag_out.opt()],
                    )

            with tc.If(pid == 0):

                def chain():
                    # static gate tiles: L0..L6 from AG_A slots of cores 1..7,
                    # L7 from AG_B core 1, L8 from AG_B core 2
                    gst = []
                    for k in range(L):
                        g = gstp.tile([P, 8], f32, tag="g")
                        if k <= 6:
                            src = ag_outA[(k + 1) * P : (k + 2) * P, :]
                        else:
                            src = ag_outB[(k - 6) * P : (k - 5) * P, :]
                        nc.sync.dma_start(g[:], src)
                        gst.append(g)

                    prev_h = None
                    inp = None
                    for k in range(L):
                        if k > 0:
                            w = wdpool.tile([P, 2048], f32, tag="wd")
                            nc.sync.dma_start(w[:], wdyn_d[k - 1, :, :])
                            G = psum.tile([P, 8], f32, tag="G")
                            if col_tile:
                                cw = P // col_tile
                                for mc in range(8):
                                    for kc in range(2):
                                        for j in range(col_tile):
                                            base = kc * 1024 + mc * P + j * cw
                                            nc.tensor.matmul(
                                                G[j * cw : (j + 1) * cw, mc : mc + 1],
                                                w[:, base : base + cw],
                                                inp[:, kc : kc + 1],
                                                start=(kc == 0),
                                                stop=(kc == 1),
                                                tile_position=(0, j * cw),
                                            )
                            else:
                                for mc in range(8):
                                    for kc in range(2):
                                        nc.tensor.matmul(
                                            G[:, mc : mc + 1],
                                            w[:, kc * 1024 + mc * P : kc * 1024 + (mc + 1) * P],
                                            inp[:, kc : kc + 1],
                                            start=(kc == 0),
                                            stop=(kc == 1),
                                        )
                            Gb = pw.tile([P, 8], f32, tag="Gb")
                            nc.vector.tensor_add(Gb[:], G[:], gst[k][:])
                        else:
                            Gb = gst[0]

                        cnew = cs_acc[:, 2 * k : 2 * k + 2]
                        hnew = hs_acc[:, 2 * k : 2 * k + 2]
                        if prescale:
                            # weights/biases for i,f,o pre-scaled by 0.5 host-side:
                            # sigmoid(2*z) gives sigma(i,f,o) and (tanh(g)+1)/2
                            S8 = pw.tile([P, 8], f32, tag="S8")
                            nc.scalar.activation(S8[:], Gb[:], ACT.Sigmoid, scale=2.0)
                            t1 = pw.tile([P, 2], f32, tag="t1")
                            t2 = pw.tile([P, 2], f32, tag="t2")
                            tm = pw.tile([P, 2], f32, tag="tm")
                            nc.vector.tensor_mul(t2[:], S8[:, 0:2], S8[:, 6:8])
                            nc.vector.tensor_mul(t1[:], S8[:, 2:4], c_in[:, 2 * k : 2 * k + 2])
                            nc.vector.scalar_tensor_tensor(
                                tm[:], t2[:], 2.0, t1[:],
                                mybir.AluOpType.mult, mybir.AluOpType.add,
                            )
                            nc.vector.tensor_sub(cnew, tm[:], S8[:, 0:2])
                            Tc = pw.tile([P, 2], f32, tag="Tc")
                            nc.scalar.activation(Tc[:], cnew, ACT.Tanh)
                            nc.vector.tensor_mul(hnew, S8[:, 4:6], Tc[:])
                        else:
                            S = pw.tile([P, 6], f32, tag="S")
                            T = pw.tile([P, 2], f32, tag="T")
                            nc.scalar.activation(S[:], Gb[:, 0:6], ACT.Sigmoid)
                            nc.scalar.activation(T[:], Gb[:, 6:8], ACT.Tanh)
                            t1 = pw.tile([P, 2], f32, tag="t1")
                            t2 = pw.tile([P, 2], f32, tag="t2")
                            nc.vector.tensor_mul(t1[:], S[:, 2:4], c_in[:, 2 * k : 2 * k + 2])
                            nc.vector.tensor_mul(t2[:], S[:, 0:2], T[:])
                            nc.vector.tensor_add(cnew, t1[:], t2[:])
                            Tc = pw.tile([P, 2], f32, tag="Tc")
                            nc.scalar.activation(Tc[:], cnew, ACT.Tanh)
                            nc.vector.tensor_mul(hnew, S[:, 4:6], Tc[:])

                        if k == 0:
                            inp = hs_acc[:, 0:2]
                        else:
                            ninp = pw.tile([P, 2], f32, tag=f"inp{k & 1}")
                            nc.vector.tensor_add(ninp[:], hnew, prev_h)
                            inp = ninp
                        prev_h = hnew

                    o_ps = pso.tile([1, 1], f32, tag="ops")
                    for kc in range(2):
                        nc.tensor.matmul(
                            o_ps[:],
                            hs_acc[:, 2 * (L - 1) + kc : 2 * (L - 1) + kc + 1],
                            wo_t[:, kc : kc + 1],
                            start=(kc == 0),
                            stop=(kc == 1),
                        )
                    o_sb = pw.tile([1, 1], f32, tag="osb")
                    nc.vector.tensor_add(o_sb[:], o_ps[:], bo_t[:])

                    nc.sync.dma_start(hs_d[:], hs_acc[:])
                    nc.sync.dma_start(cs_d[:], cs_acc[:])
                    nc.sync.dma_start(out_d[:], o_sb[:])

                if reps > 0:
                    with tc.For_i(0, reps, 1):
                        chain()
                else:
                    chain()

    nc.compile()
    return nc


def _prep_inputs_v3(x, h, c, W_ih1, W_ih, W_hh, b_ih, b_hh, W_out, b_out, prescale=False):
    """Per-core inputs for _build_v3."""
    base = _prep_inputs(x, h, c, W_ih1, W_ih, W_hh, b_ih, b_hh, W_out, b_out)
    x = np.asarray(x, F32).reshape(2)
    W_ih = np.asarray(W_ih, F32)
    W_ih1 = np.asarray(W_ih1, F32)
    W_hh = np.asarray(W_hh, F32)
    b_raw = np.asarray(b_ih, F32) + np.asarray(b_hh, F32)  # [9, 1024]
    b_eff = b_raw.copy()
    b_eff[0] += W_ih1 @ x  # fold the tiny input matvec into layer-0 bias
    h2 = np.asarray(h, F32).reshape(L, H)

    sc = 0.5 if prescale else 1.0  # i,f,o rows (perm'd 0:768) pre-scaled

    def _scaled(v):
        v = v.copy()
        v[0:768] *= sc
        return v

    wdyn = np.zeros((L - 1, P, 2048), F32)
    for k in range(1, L):
        wp = _scaled(W_ih[k - 1][_PERM, :])  # [1024, 256]
        wdyn[k - 1] = wp.T.reshape(2, P, 8, P).transpose(1, 0, 2, 3).reshape(P, 2048)

    def stat_w(k):
        wp = _scaled(W_hh[k][_PERM, :])
        return np.ascontiguousarray(
            wp.T.reshape(2, P, 8, P).transpose(1, 0, 2, 3).reshape(P, 2048)
        )

    def stat_b(k):
        return np.ascontiguousarray(_scaled(b_eff[k][_PERM]).reshape(8, P).T)

    # slot A of core c -> layer c-1 (L0..L6); slot B: core1 -> L7, core2 -> L8
    slot_map = {c: [c - 1, None] for c in range(1, 8)}
    slot_map[1][1] = 7
    slot_map[2][1] = 8

    maps = []
    for cid in range(N_CORES):
        m = {
            "wdyn": wdyn if cid == 0 else np.zeros_like(wdyn),
            "h0": base["h0"],
            "c0": base["c0"],
            "x": base["x"],
            "wo": base["wo"],
            "bo": base["bo"],
        }
        wst = np.zeros((2, P, 2048), F32)
        bst = np.zeros((2, P, 8), F32)
        hsl = np.zeros((2, P, 2), F32)
        if cid > 0:
            for s, k in enumerate(slot_map[cid]):
                if k is None:
                    continue
                wst[s] = stat_w(k)
                bst[s] = stat_b(k)
                hsl[s] = _vec_to_pf(h2[k])
        m["wst"], m["bst"], m["hslot"] = wst, bst, hsl
        maps.append(m)
    return maps


def _build_v2(reps=0, with_ag=True, mm_mc=8, slim_pw=False, transposed=False, split_psum=False):
    """Distributed build: cores 1-7 compute static gates (W_hh@h[k]+b) for
    layers 2-8 in [128,8] form, one AllGather ships them to core 0, which runs
    layers 0-1 full-style plus the dynamic chain for layers 2-8.
    reps>0 (benchmark mode) requires with_ag=False (collectives can't sit in
    control flow); the chain then reads its local (garbage) ag_out."""
    import concourse.bacc as bacc
    import concourse.mybir as mybir
    import concourse.tile as tile

    assert reps == 0 or not with_ag
    f32 = mybir.dt.float32
    ACT = mybir.ActivationFunctionType
    nc = bacc.Bacc(
        name=f"decoder_v2_r{reps}_{int(with_ag)}_{mm_mc}_{int(slim_pw)}_{int(transposed)}_{int(split_psum)}",
        num_devices=N_CORES,
    )

    wcat01_d = nc.dram_tensor("wcat01", [2, P, 4096], f32, kind="ExternalInput")
    wdyn_d = nc.dram_tensor("wdyn", [L - 2, P, 2048], f32, kind="ExternalInput")
    wst_d = nc.dram_tensor("wst", [P, 2048], f32, kind="ExternalInput")
    bst_d = nc.dram_tensor("bst", [P, 8], f32, kind="ExternalInput")
    hslot_d = nc.dram_tensor("hslot", [P, 2], f32, kind="ExternalInput")
    b_d = nc.dram_tensor("b", [P, L * 8], f32, kind="ExternalInput")
    h_d = nc.dram_tensor("h0", [P, L * 2], f32, kind="ExternalInput")
    c_d = nc.dram_tensor("c0", [P, L * 2], f32, kind="ExternalInput")
    x_d = nc.dram_tensor("x", [P, 2], f32, kind="ExternalInput")
    wo_d = nc.dram_tensor("wo", [P, 2], f32, kind="ExternalInput")
    bo_d = nc.dram_tensor("bo", [1, 1], f32, kind="ExternalInput")

    hs_d = nc.dram_tensor("hs", [P, L * 2], f32, kind="ExternalOutput")
    cs_d = nc.dram_tensor("cs", [P, L * 2], f32, kind="ExternalOutput")
    out_d = nc.dram_tensor("out", [1, 1], f32, kind="ExternalOutput")

    with tile.TileContext(nc) as tc:
        with (
            tc.tile_pool(name="wpool", bufs=3) as wpool,
            tc.tile_pool(name="wdpool", bufs=4) as wdpool,
            tc.tile_pool(name="small", bufs=1) as small,
            tc.tile_pool(name="gst", bufs=L - 2) as gstp,
            tc.tile_pool(name="pw", bufs=2) as pw,
            tc.tile_pool(name="psum", bufs=2, space="PSUM") as psum,
            tc.tile_pool(name="pso", bufs=1, space="PSUM") as pso,
            tc.tile_pool(name="dram", bufs=1, space="DRAM") as dram,
        ):
            h_in = small.tile([P, L * 2], f32, tag="h_in")
            c_in = small.tile([P, L * 2], f32, tag="c_in")
            x_t = small.tile([P, 2], f32, tag="x_t")
            b_t = small.tile([P, L * 8], f32, tag="b_t")
            wo_t = small.tile([P, 2], f32, tag="wo_t")
            bo_t = small.tile([1, 1], f32, tag="bo_t")
            hs_acc = small.tile([P, L * 2], f32, tag="hs_acc")
            cs_acc = small.tile([P, L * 2], f32, tag="cs_acc")
            dummy = small.tile([P, 1], f32, tag="dummy")

            nc.sync.dma_start(h_in[:], h_d[:])
            nc.sync.dma_start(c_in[:], c_d[:])
            nc.sync.dma_start(x_t[:], x_d[:])
            nc.sync.dma_start(b_t[:], b_d[:])
            nc.sync.dma_start(wo_t[:], wo_d[:])
            nc.sync.dma_start(bo_t[:], bo_d[:])
            nc.scalar.activation(dummy[:], x_t[:, 0:1], ACT.Sigmoid)

            ag_in = dram.tile([P, 8], f32, tag="ag_in")
            ag_out = dram.tile([N_CORES * P, 8], f32, tag="ag_out")

            pid = nc.partition_id()

            # ---- producers: static gates for layer (pid+1), pid=1..7 ----
            with tc.If(pid != 0):
                wst = small.tile([P, 2048], f32, tag="wst")
                bst = small.tile([P, 8], f32, tag="bst")
                hsl = small.tile([P, 2], f32, tag="hsl")
                nc.sync.dma_start(wst[:], wst_d[:])
                nc.sync.dma_start(bst[:], bst_d[:])
                nc.sync.dma_start(hsl[:], hslot_d[:])
                Gs = psum.tile([P, 8], f32, tag="Gs")
                for mc in range(8):
                    for kc in range(2):
                        nc.tensor.matmul(
                            Gs[:, mc : mc + 1],
                            wst[:, kc * 1024 + mc * P : kc * 1024 + (mc + 1) * P],
                            hsl[:, kc : kc + 1],
                            start=(kc == 0),
                            stop=(kc == 1),
                        )
                Gsb = pw.tile([P, 8], f32, tag="Gsb")
                nc.vector.tensor_add(Gsb[:], Gs[:], bst[:])
                nc.sync.dma_start(ag_in[:], Gsb[:])

            if with_ag:
                nc.gpsimd.collective_compute(
                    "AllGather",
                    mybir.AluOpType.bypass,
                    replica_groups=[list(range(N_CORES))],
                    ins=[ag_in.opt()],
                    outs=[ag_out.opt()],
                )

            # ---- core 0: the chain ----
            with tc.If(pid == 0):

                def chain():
                    # prefetch static gate tiles for layers 2..8 (slots 1..7)
                    gst = []
                    for j in range(L - 2):
                        g = gstp.tile([P, 8], f32, tag="g")
                        nc.sync.dma_start(g[:], ag_out[(j + 1) * P : (j + 2) * P, :])
                        gst.append(g)

                    prev_h = None
                    inp = None
                    for k in range(L):
                        if k < 2:
                            w = wpool.tile([P, 4096], f32, tag="w")
                            nc.sync.dma_start(w[:], wcat01_d[k, :, :])
                        else:
                            w = wdpool.tile([P, 2048], f32, tag="wd")
                            nc.sync.dma_start(w[:], wdyn_d[k - 2, :, :])

                        nkc = 4 if k < 2 else 2

                        def vec(kc):
                            if k < 2:
                                if kc < 2:
                                    return h_in[:, 2 * k + kc : 2 * k + kc + 1]
                                elif k == 0:
                                    return x_t[:, kc - 2 : kc - 1]
                                else:
                                    return inp[:, kc - 2 : kc - 1]
                            return inp[:, kc : kc + 1]

                        if transposed:
                            # gates^T in two [1,512] psum halves; weights stream as rhs
                            g1s = pw.tile([1, 1024], f32, tag="g1s")
                            for nh in range(2):
                                Gh = psum.tile([1, 512], f32, tag="Gh")
                                for kc in range(nkc):
                                    nc.tensor.matmul(
                                        Gh[:],
                                        vec(kc),
                                        w[:, kc * 1024 + nh * 512 : kc * 1024 + nh * 512 + 512],
                                        start=(kc == 0),
                                        stop=(kc == nkc - 1),
                                    )
                                nc.scalar.activation(
                                    g1s[:, nh * 512 : (nh + 1) * 512], Gh[:], ACT.Copy
                                )
                            G = pw.tile([P, 8], f32, tag="G8")
                            nc.sync.dma_start(G[:], g1s[:, :])
                        elif split_psum:
                            # alternate accumulation groups across two PSUM banks
                            Ga = psum.tile([P, 4], f32, tag="Ga")
                            Gb2 = psum.tile([P, 4], f32, tag="Gb2")
                            for step in range(mm_mc):
                                half, col = step % 2, step // 2
                                tgt = Ga if half == 0 else Gb2
                                mc = half * 4 + col
                                for kc in range(nkc):
                                    nc.tensor.matmul(
                                        tgt[:, col : col + 1],
                                        w[:, kc * 1024 + mc * P : kc * 1024 + (mc + 1) * P],
                                        vec(kc),
                                        start=(kc == 0),
                                        stop=(kc == nkc - 1),
                                    )
                            G = None
                        else:
                            G = psum.tile([P, 8], f32, tag="G")
                            for mc in range(mm_mc):
                                for kc in range(nkc):
                                    nc.tensor.matmul(
                                        G[:, mc : mc + 1],
                                        w[:, kc * 1024 + mc * P : kc * 1024 + (mc + 1) * P],
                                        vec(kc),
                                        start=(kc == 0),
                                        stop=(kc == nkc - 1),
                                    )

                        Gb = pw.tile([P, 8], f32, tag="Gb")
                        statics = b_t[:, 8 * k : 8 * k + 8] if k < 2 else gst[k - 2][:]
                        if split_psum and not transposed:
                            nc.vector.tensor_add(Gb[:, 0:4], Ga[:], statics[:, 0:4])
                            nc.vector.tensor_add(Gb[:, 4:8], Gb2[:], statics[:, 4:8])
                        else:
                            nc.vector.tensor_add(Gb[:], G[:], statics)
                        if slim_pw:
                            Sx = pw.tile([P, 8], f32, tag="Sx")
                            nc.scalar.activation(Sx[:], Gb[:], ACT.Sigmoid)
                            cnew = cs_acc[:, 2 * k : 2 * k + 2]
                            nc.vector.tensor_mul(cnew, Sx[:, 2:4], c_in[:, 2 * k : 2 * k + 2])
                            hnew = hs_acc[:, 2 * k : 2 * k + 2]
                            nc.vector.tensor_mul(hnew, Sx[:, 4:6], cnew)
                            if k == 0:
                                inp = hs_acc[:, 0:2]
                            else:
                                ninp = pw.tile([P, 2], f32, tag=f"inp{k & 1}")
                                nc.vector.tensor_add(ninp[:], hnew, prev_h)
                                inp = ninp
                            prev_h = hnew
                            continue
                        S = pw.tile([P, 6], f32, tag="S")
                        T = pw.tile([P, 2], f32, tag="T")
                        nc.scalar.activation(S[:], Gb[:, 0:6], ACT.Sigmoid)
                        nc.scalar.activation(T[:], Gb[:, 6:8], ACT.Tanh)
                        t1 = pw.tile([P, 2], f32, tag="t1")
                        t2 = pw.tile([P, 2], f32, tag="t2")
                        nc.vector.tensor_mul(t1[:], S[:, 2:4], c_in[:, 2 * k : 2 * k + 2])
                        nc.vector.tensor_mul(t2[:], S[:, 0:2], T[:])
                        cnew = cs_acc[:, 2 * k : 2 * k + 2]
                        nc.vector.tensor_add(cnew, t1[:], t2[:])
                        Tc = pw.tile([P, 2], f32, tag="Tc")
                        nc.scalar.activation(Tc[:], cnew, ACT.Tanh)
                        hnew = hs_acc[:, 2 * k : 2 * k + 2]
                        nc.vector.tensor_mul(hnew, S[:, 4:6], Tc[:])

                        if k == 0:
                            inp = hs_acc[:, 0:2]
                        else:
                            ninp = pw.tile([P, 2], f32, tag=f"inp{k & 1}")
                            nc.vector.tensor_add(ninp[:], hnew, prev_h)
                            inp = ninp
                        prev_h = hnew

                    o_ps = pso.tile([1, 1], f32, tag="ops")
                    for kc in range(2):
                        nc.tensor.matmul(
                            o_ps[:],
                            hs_acc[:, 2 * (L - 1) + kc : 2 * (L - 1) + kc + 1],
                            wo_t[:, kc : kc + 1],
                            start=(kc == 0),
                            stop=(kc == 1),
                        )
                    o_sb = pw.tile([1, 1], f32, tag="osb")
                    nc.vector.tensor_add(o_sb[:], o_ps[:], bo_t[:])

                    nc.sync.dma_start(hs_d[:], hs_acc[:])
                    nc.sync.dma_start(cs_d[:], cs_acc[:])
                    nc.sync.dma_start(out_d[:], o_sb[:])

                if reps > 0:
                    with tc.For_i(0, reps, 1):
                        chain()
                else:
                    chain()

    nc.compile()
    return nc


def _prep_inputs_v2(x, h, c, W_ih1, W_ih, W_hh, b_ih, b_hh, W_out, b_out, transposed=False):
    """Returns list of 8 per-core input dicts."""
    base = _prep_inputs(x, h, c, W_ih1, W_ih, W_hh, b_ih, b_hh, W_out, b_out)
    W_ih = np.asarray(W_ih, F32)
    W_ih1 = np.asarray(W_ih1, F32)
    W_hh = np.asarray(W_hh, F32)
    b_raw = np.asarray(b_ih, F32) + np.asarray(b_hh, F32)  # [9, 1024] unpermuted
    b = b_raw[:, _PERM]
    h2 = np.asarray(h, F32).reshape(L, H)

    if transposed:
        # moving-weight layout: w[p, kc*1024 + nh*512 + n] = W[gperm[nh*512+n], kc*128+p]
        wcat01 = np.zeros((2, P, 4096), F32)
        for k in range(2):
            wcat = np.zeros((NG, 2 * H), F32)
            wcat[:, :H] = W_hh[k]
            if k == 0:
                wcat[:, H : H + 2] = W_ih1
            else:
                wcat[:, H:] = W_ih[0]
            wp = wcat[_GPERM_STREAM, :]  # [1024, 512]
            wcat01[k] = wp.T.reshape(4, P, 2, 512).transpose(1, 0, 2, 3).reshape(P, 4096)
        wdyn = np.zeros((L - 2, P, 2048), F32)
        for k in range(2, L):
            wp = W_ih[k - 1][_GPERM_STREAM, :]  # [1024, 256]
            wdyn[k - 2] = wp.T.reshape(2, P, 2, 512).transpose(1, 0, 2, 3).reshape(P, 2048)
        base = dict(base)
        barr = b_raw[:, _GMAP]  # [9, 128, 8]
        base["b"] = np.ascontiguousarray(barr.transpose(1, 0, 2).reshape(P, L * 8))
    else:
        wcat01 = base["w"][0:2]  # [2,128,4096]
        wdyn = np.zeros((L - 2, P, 2048), F32)
        for k in range(2, L):
            wp = W_ih[k - 1][_PERM, :]  # [1024, 256]
            wdyn[k - 2] = wp.T.reshape(2, P, 8, P).transpose(1, 0, 2, 3).reshape(P, 2048)

    zeros_wst = np.zeros((P, 2048), F32)
    zeros_b8 = np.zeros((P, 8), F32)
    zeros_h2 = np.zeros((P, 2), F32)
    maps = []
    for cid in range(N_CORES):
        m = {
            "wcat01": wcat01 if cid == 0 else np.zeros_like(wcat01),
            "wdyn": wdyn if cid == 0 else np.zeros_like(wdyn),
            "b": base["b"],
            "h0": base["h0"],
            "c0": base["c0"],
            "x": base["x"],
            "wo": base["wo"],
            "bo": base["bo"],
        }
        if cid == 0:
            m["wst"] = zeros_wst
            m["bst"] = zeros_b8
            m["hslot"] = zeros_h2
        else:
            k = cid + 1  # layer produced by this core
            rowsel = _GPERM_STAT if transposed else _PERM
            wp = W_hh[k][rowsel, :]  # [1024, 256]
            m["wst"] = np.ascontiguousarray(
                wp.T.reshape(2, P, 8, P).transpose(1, 0, 2, 3).reshape(P, 2048)
            )
            if transposed:
                m["bst"] = np.ascontiguousarray(b_raw[k][_GMAP])
            else:
                m["bst"] = np.ascontiguousarray(b[k].reshape(8, P).T)
            m["hslot"] = _vec_to_pf(h2[k])
        maps.append(m)
    return maps


def _prep_inputs(x, h, c, W_ih1, W_ih, W_hh, b_ih, b_hh, W_out, b_out):
    x = np.asarray(x, F32).reshape(2)
    h = np.asarray(h, F32).reshape(L, H)
    c = np.asarray(c, F32).reshape(L, H)
    W_ih1 = np.asarray(W_ih1, F32)
    W_ih = np.asarray(W_ih, F32)
    W_hh = np.asarray(W_hh, F32)
    b = (np.asarray(b_ih, F32) + np.asarray(b_hh, F32))[:, _PERM]  # [9, 1024]
    W_out = np.asarray(W_out, F32).reshape(1, H)
    b_out = np.asarray(b_out, F32).reshape(1)

    w_host = np.zeros((L, P, 4096), F32)
    for k in range(L):
        wcat = np.zeros((NG, 2 * H), F32)
        wcat[:, :H] = W_hh[k]
        if k == 0:
            wcat[:, H : H + 2] = W_ih1
        else:
            wcat[:, H:] = W_ih[k - 1]
        wp = wcat[_PERM, :]  # [1024, 512]
        lt = wp.T.reshape(4, P, 8, P)  # [kc, p, mc, pm]
        w_host[k] = lt.transpose(1, 0, 2, 3).reshape(P, 4096)

    xp = np.zeros(H, F32)
    xp[:2] = x

    return {
        "w": w_host,
        "b": np.ascontiguousarray(b.reshape(L, 8, P).transpose(2, 0, 1).reshape(P, L * 8)),
        "h0": _vec_to_pf(h),
        "c0": _vec_to_pf(c),
        "x": _vec_to_pf(xp),
        "wo": _vec_to_pf(W_out[0]),
        "bo": b_out.reshape(1, 1),
    }


def _postprocess(r0):
    hs = np.asarray(r0["hs"])  # [128, 18]
    cs = np.asarray(r0["cs"])
    out = np.asarray(r0["out"]).reshape(1, 1)
    hs_full = hs.reshape(P, L, 2).transpose(1, 2, 0).reshape(L, 1, 1, H)
    cs_full = cs.reshape(P, L, 2).transpose(1, 2, 0).reshape(L, 1, 1, H)
    return (
        np.ascontiguousarray(out),
        np.ascontiguousarray(hs_full),
        np.ascontiguousarray(cs_full),
    )


def make_runner(reps=0, ver="v1", with_ag=True, mm_mc=8, slim_pw=False, transposed=False, split_psum=False):
    """Build (cached) and return run(in_maps_list) -> list of per-core result dicts.
    Keeps a persistent jitted callable so repeated calls avoid retracing."""
    import jax
    import numpy as _np
    from jax.sharding import Mesh, PartitionSpec
    from jax.experimental.shard_map import shard_map
    import concourse.mybir as mybir
    from concourse import bass2jax

    key = ("runner", reps, ver, with_ag, mm_mc, slim_pw, transposed, split_psum)
    if key in _CACHE:
        return _CACHE[key]

    if ver == "v1":
        nc = _build(reps)
    elif ver == "v3":
        nc = _build_v3(
            reps, with_ag, col_tile=mm_mc if mm_mc in (2, 4) else 0, prescale=slim_pw
        )
    else:
        nc = _build_v2(reps, with_ag, mm_mc, slim_pw, transposed, split_psum)
    bass2jax.install_neuronx_cc_hook()

    partition_name = nc.partition_id_tensor.name if nc.partition_id_tensor else None
    in_names, out_names, out_avals, zero_outs = [], [], [], []
    for alloc in nc.m.functions[0].allocations:
        if not isinstance(alloc, mybir.MemoryLocationSet):
            continue
        name = alloc.memorylocations[0].name
        if alloc.kind == "ExternalInput":
            if name != partition_name:
                in_names.append(name)
        elif alloc.kind == "ExternalOutput":
            shape = tuple(alloc.tensor_shape)
            dtype = mybir.dt.np(alloc.dtype)
            out_names.append(name)
            out_avals.append(jax.core.ShapedArray(shape, dtype))
            zero_outs.append(_np.zeros(shape, dtype))
    n_params = len(in_names)
    n_outs = len(out_avals)
    all_in_names = in_names + out_names + ([partition_name] if partition_name else [])
    donate = tuple(range(n_params, n_params + n_outs))

    def _body(*args):
        operands = list(args)
        if partition_name is not None:
            operands.append(bass2jax.partition_id_tensor())
        outs = bass2jax._bass_exec_p.bind(
            *operands,
            out_avals=tuple(out_avals),
            in_names=tuple(all_in_names),
            out_names=tuple(out_names),
            lowering_input_output_aliases=(),
            sim_require_finite=True,
            sim_require_nnan=True,
            nc=nc,
        )
        return tuple(outs)

    devices = jax.devices()[:N_CORES]
    mesh = Mesh(_np.asarray(devices), ("core",))
    in_specs = (PartitionSpec("core"),) * (n_params + n_outs)
    out_specs = (PartitionSpec("core"),) * n_outs
    sharded = jax.jit(
        shard_map(_body, mesh=mesh, in_specs=in_specs, out_specs=out_specs, check_rep=False),
        donate_argnums=donate,
        keep_unused=True,
    )

    state = {}

    def run(in_maps, reuse_inputs=False):
        if not reuse_inputs or "dev_in" not in state:
            concat_in = [
                _np.concatenate([_np.asarray(m[nm]) for m in in_maps], axis=0)
                for nm in in_names
            ]
            state["dev_in"] = [jax.device_put(a) for a in concat_in]
        concat_zeros = [
            _np.zeros((N_CORES * z.shape[0], *z.shape[1:]), z.dtype) for z in zero_outs
        ]
        out_arrs = sharded(*state["dev_in"], *concat_zeros)
        jax.block_until_ready(out_arrs)
        return [
            {
                nm: _np.asarray(out_arrs[i]).reshape(N_CORES, *out_avals[i].shape)[c]
                for i, nm in enumerate(out_names)
            }
            for c in range(N_CORES)
        ], out_arrs

    _CACHE[key] = run
    return run


def kernel(x, h, c, W_ih1, W_ih, W_hh, b_ih, b_hh, W_out, b_out):
    in_maps = _prep_inputs_v3(x, h, c, W_ih1, W_ih, W_hh, b_ih, b_hh, W_out, b_out)
    run = make_runner(0, ver="v3", with_ag=True, mm_mc=2)
    results, _ = run(in_maps)
    return _postprocess(results[0])
